# revision 26
# baseline (speedup 1.0000x reference)
"""Trainium2 Bass kernel for nn_Decoder (Tacotron2-style decoder with
forward attention), SPMD across 8 NeuronCores.

Strategy:
  - Tensor-parallel over LSTM hidden units: core c owns hidden slice
    [128c:128c+128) of both LSTMs (gate rows reordered [i f o g]).
    Weights stay resident in SBUF; per-step AllGathers share the
    transposed hidden states.
  - Attention / projection / outputs are batch-sharded: core c owns
    batches [4c:4c+4).
  - 3 AllGathers per step: ah (after attention-LSTM), ctx (after
    attention), dh (after decoder-LSTM).
  - Prenet (jax PRNG dropout) is computed on host; everything else on
    device.

kernel(memory, decoder_inputs, params) -> (mel_outputs, gate_outputs,
alignments), matching the reference's return structure.
"""

import numpy as np

B, T_IN, T_OUT = 32, 200, 500
NMEL, E, ARNN, DRNN, ADIM = 80, 832, 1024, 1024, 128
NFILT, KS, PRE = 32, 31, 256
ZO = 0.1  # zoneout
NEG = -1e20
NCORES = 8
HS = ARNN // NCORES      # 128: hidden slice per core
GS = 4 * HS              # 512: gate rows per core
BS = B // NCORES         # 4: batch slice per core
KC = 7                   # ctx k-tiles (832 -> 7x128, last padded)
KH = 8                   # hidden k-tiles (1024)
KX = 2                   # prenet k-tiles (256)
KP = 15                  # proj k-tiles (1024 + 832 -> 8 + 7)
NO = NMEL + 1            # 81: mel + gate columns

_CACHE = {}
LAST_RESULT = None


# ----------------------------------------------------------------- host prep

def _prenet_host(decoder_inputs, params):
    """Exact replica of reference prenet (jax threefry dropout)."""
    import jax
    import jax.numpy as jnp
    cpu = jax.devices("cpu")[0]
    with jax.default_device(cpu):
        di = jnp.asarray(decoder_inputs)
        go = jnp.zeros((1, B, NMEL), di.dtype)
        din = jnp.concatenate([go, jnp.transpose(di, (2, 0, 1))], 0)[:T_OUT]
        x = din
        key = jax.random.key(123)
        for idx, wn in enumerate(["pre_w1", "pre_w2"]):
            w = jnp.asarray(params[wn])
            x = jax.nn.relu(x @ w.T)
            mask = jax.random.bernoulli(jax.random.fold_in(key, idx), 0.5, x.shape)
            x = jnp.where(mask, x * 2.0, 0.0)
        return np.asarray(x, np.float32)  # [T_OUT, B, PRE]


def _ktile(w, k_pad=None):
    """[rows, K] -> [128, ceil(K/128), rows] laid out for rhs tiles.

    Returns arr[p, kt, g] = w[g, 128*kt + p], zero-padded in K."""
    rows, K = w.shape
    kt = (K + 127) // 128 if k_pad is None else k_pad
    wp = np.zeros((rows, kt * 128), np.float32)
    wp[:, :K] = w
    return np.ascontiguousarray(wp.T.reshape(kt, 128, rows).transpose(1, 0, 2))


def _prepare_in_maps(memory, decoder_inputs, params):
    p = {k: np.asarray(v, np.float32) for k, v in params.items()}
    memory = np.asarray(memory, np.float32)

    pren = _prenet_host(decoder_inputs, params)  # [T, B, 256]
    # pren layout [T, 128, KX, B]
    pren_l = np.zeros((T_OUT, 128, KX, B), np.float32)
    for k in range(KX):
        pren_l[:, :, k, :] = pren[:, :, 128 * k:128 * (k + 1)].transpose(0, 2, 1)

    pm_full = np.einsum("bte,ae->bta", memory, p["wm"])  # [B, T_in, ADIM]

    # fused location conv x loc_dense: wf[a, c, k]
    wf = np.einsum("af,fck->ack", p["loc_dense"], p["loc_conv"])
    wf_T = np.ascontiguousarray(wf.reshape(ADIM, 2 * KS).T)  # [62, 128]

    v = p["v"][0]  # [128]
    v_diag = np.zeros((ADIM, BS, BS), np.float32)
    for b in range(BS):
        v_diag[:, b, b] = v

    wq_T = _ktile(p["wq"], KH)  # [128, 8, 128]

    W_pg = np.concatenate([p["proj_w"], p["gate_w"]], 0)  # [81, 1856]
    wp_dh = _ktile(W_pg[:, :DRNN], KH)       # [128, 8, 81]
    wp_ctx = _ktile(W_pg[:, DRNN:], KC)      # [128, 7, 81]
    wp_l = np.concatenate([wp_dh, wp_ctx], 1)  # [128, 15, 81]
    wp_b = np.zeros((128, NO), np.float32)
    wp_b[0, :NMEL] = p["proj_b"]
    wp_b[0, NMEL:] = p["gate_b"]

    ones_row = np.zeros((128, B), np.float32)
    ones_row[0] = 1.0

    in_maps = []
    for c in range(NCORES):
        rows = np.r_[0 * ARNN + HS * c: 0 * ARNN + HS * (c + 1),
                     1 * ARNN + HS * c: 1 * ARNN + HS * (c + 1),
                     3 * ARNN + HS * c: 3 * ARNN + HS * (c + 1),
                     2 * ARNN + HS * c: 2 * ARNN + HS * (c + 1)]  # i f o g
        wa = p["arnn_wih"][rows]     # [512, 1088]
        wd = p["drnn_wih"][rows]     # [512, 1856]
        wah = p["arnn_whh"][rows]    # [512, 1024]
        wdh = p["drnn_whh"][rows]
        bias_tile = np.zeros((128, 2, GS), np.float32)
        bias_tile[0, 0] = p["arnn_b"][rows]
        bias_tile[0, 1] = p["drnn_b"][rows]

        bsl = slice(BS * c, BS * (c + 1))
        mem_b = memory[bsl]  # [4, 200, 832]
        mem_T = np.zeros((BS, 2, 128, E), np.float32)
        mem_T[:, 0] = mem_b[:, :128]
        mem_T[:, 1, :T_IN - 128] = mem_b[:, 128:]
        pm_l = np.ascontiguousarray(pm_full[bsl].transpose(2, 0, 1))  # [128,4,200]

        in_maps.append({
            "pren": pren_l,
            "wa_x": _ktile(wa[:, :PRE], KX),
            "wa_ctx": _ktile(wa[:, PRE:], KC),
            "wa_h": _ktile(wah, KH),
            "wd_ah": _ktile(wd[:, :ARNN], KH),
            "wd_ctx": _ktile(wd[:, ARNN:], KC),
            "wd_h": _ktile(wdh, KH),
            "bias_tile": bias_tile,
            "ones_row": ones_row,
            "wq_T": wq_T,
            "wf_T": wf_T,
            "v_diag": v_diag,
            "wp": wp_l,
            "wp_b": wp_b,
            "mem_T": mem_T,
            "pm": pm_l,
        })
    return in_maps


# ---------------------------------------------------------------- bass build

def _build(t_steps):
    import concourse.bass as bass
    import concourse.mybir as mybir
    import concourse.tile as tile
    from concourse import bacc
    from concourse.bass import AP
    from concourse.masks import make_identity

    f32 = mybir.dt.float32
    AF = mybir.ActivationFunctionType
    OP = mybir.AluOpType

    nc = bacc.Bacc("TRN2", target_bir_lowering=False, debug=False,
                   enable_asserts=False, num_devices=NCORES)

    din = {}
    for name, shape in [
        ("pren", [T_OUT, 128, KX, B]), ("wa_x", [128, KX, GS]),
        ("wa_ctx", [128, KC, GS]), ("wa_h", [128, KH, GS]),
        ("wd_ah", [128, KH, GS]), ("wd_ctx", [128, KC, GS]),
        ("wd_h", [128, KH, GS]), ("bias_tile", [128, 2, GS]),
        ("ones_row", [128, B]), ("wq_T", [128, KH, ADIM]),
        ("wf_T", [2 * KS, 128]), ("v_diag", [ADIM, BS, BS]),
        ("wp", [128, KP, NO]), ("wp_b", [128, NO]),
        ("mem_T", [BS, 2, 128, E]), ("pm", [ADIM, BS, T_IN]),
    ]:
        din[name] = nc.dram_tensor(name, shape, f32, kind="ExternalInput")

    mels_o = nc.dram_tensor("mels_o", [T_OUT, BS, NMEL], f32, kind="ExternalOutput")
    gates_o = nc.dram_tensor("gates_o", [T_OUT, BS], f32, kind="ExternalOutput")
    aligns_o = nc.dram_tensor("aligns_o", [T_OUT, BS, T_IN], f32, kind="ExternalOutput")

    # dram scratch for conv input (overlapping-read im2col source)
    conv_d = nc.dram_tensor("conv_d", [2, BS, T_IN + KS - 1], f32, kind="Internal")

    rg = [list(range(NCORES))]

    with tile.TileContext(nc) as tc:
        with tc.tile_pool(name="const", bufs=1) as const, \
             tc.tile_pool(name="state", bufs=1) as state, \
             tc.tile_pool(name="work", bufs=3) as work, \
             tc.tile_pool(name="attw", bufs=2) as attw, \
             tc.tile_pool(name="pgates", bufs=1, space="PSUM") as pgates, \
             tc.tile_pool(name="pbig", bufs=2, space="PSUM") as pbig, \
             tc.tile_pool(name="psmall", bufs=3, space="PSUM") as psmall, \
             tc.tile_pool(name="dram", bufs=4, space="DRAM") as dram:

            # ---- constants / weights into SBUF
            cw = {}
            for name in ["wa_x", "wa_ctx", "wa_h", "wd_ah", "wd_ctx", "wd_h",
                         "bias_tile", "ones_row", "wq_T", "wf_T", "v_diag",
                         "wp", "wp_b", "pm"]:
                t = const.tile(list(din[name].shape), f32, tag=name)
                nc.sync.dma_start(t[:], din[name][:])
                cw[name] = t
            ident = const.tile([128, 128], f32, tag="ident")
            make_identity(nc, ident)
            mem_sb = const.tile([128, BS, 2, E], f32, tag="mem_sb")
            nc.sync.dma_start(mem_sb[:], din["mem_T"].rearrange("s k p e -> p s k e"))

            # ---- states
            ahT = state.tile([128, KH, B], f32, tag="ahT")    # gathered ah^T
            dhT = state.tile([128, KH, B], f32, tag="dhT")
            ctxT = state.tile([128, KC, B], f32, tag="ctxT")  # gathered ctx^T
            ah_s = state.tile([B, HS], f32, tag="ah_s")       # my slice (b-major)
            dh_s = state.tile([B, HS], f32, tag="dh_s")
            ac = state.tile([B, HS], f32, tag="ac")
            dc = state.tile([B, HS], f32, tag="dc")
            # attention state in probability domain: aw_buf[:, 1:] = exp(la - C)
            # (softmax-shift-invariant), guard col 0 = 0 for the left shift.
            aw_buf = state.tile([BS, T_IN + 1], f32, tag="aw_buf")
            w_cum = state.tile([BS, T_IN], f32, tag="w_cum")
            zz = state.tile([BS, T_IN + KS - 1], f32, tag="zz")

            # aw^T stored block-diagonal: awT_bd[p, kt, b, j] = awT_b[p, kt] if
            # j == b else 0, so the ctx einsum becomes plain K-tile
            # accumulation at base partition 0 (no tile_position needed).
            awT_bd = state.tile([128, 2, BS, BS], f32, tag="awT_bd")
            for t_ in (ahT, dhT, ctxT, ah_s, dh_s, ac, dc, w_cum, zz, awT_bd):
                nc.vector.memset(t_[:], 0.0)
            nc.vector.memset(aw_buf[:], 0.0)
            nc.vector.memset(aw_buf[:, 1:2], 1.0)
            # zero-init conv dram (incl. padding columns)
            nc.sync.dma_start(conv_d[0], zz[:])
            nc.sync.dma_start(conv_d[1], zz[:])

            # SPMD: one program for all cores -- "own batch" selection must be
            # data-driven. sel_b [B, BS] is a per-core one-hot matrix picking
            # the core's 4 batch columns (used for pq and the projection).
            sel_b = nc.dram_tensor("sel_b", [B, BS], f32, kind="ExternalInput")
            din["sel_b"] = sel_b
            sel_sb = const.tile([B, BS], f32, tag="sel_sb")
            nc.sync.dma_start(sel_sb[:], sel_b[:])

            def lstm_cell(gates_ps, c_st, h_s, pfx):
                """gates_ps [B, 512] (i f o g), updates c_st, h_s in place."""
                sio = work.tile([B, 3 * HS], f32, tag=f"{pfx}sio")
                tg = work.tile([B, HS], f32, tag=f"{pfx}tg")
                nc.scalar.activation(sio[:], gates_ps[:, 0:3 * HS], AF.Sigmoid)
                nc.scalar.activation(tg[:], gates_ps[:, 3 * HS:4 * HS], AF.Tanh)
                si = sio[:, 0:HS]
                sf = sio[:, HS:2 * HS]
                so = sio[:, 2 * HS:3 * HS]
                c2 = work.tile([B, HS], f32, tag=f"{pfx}c2")
                m2 = work.tile([B, HS], f32, tag=f"{pfx}m2")
                nc.vector.tensor_mul(c2[:], sf, c_st[:])
                nc.vector.tensor_mul(m2[:], si, tg[:])
                nc.vector.tensor_add(c2[:], c2[:], m2[:])
                th = work.tile([B, HS], f32, tag=f"{pfx}th")
                nc.scalar.activation(th[:], c2[:], AF.Tanh)
                # c_st = 0.1*c_st + 0.9*c2
                nc.vector.scalar_tensor_tensor(
                    m2[:], c2[:], 0.9 / 0.1, c_st[:], OP.mult, OP.add)
                nc.vector.tensor_scalar_mul(c_st[:], m2[:], 0.1)
                # h_s = 0.1*h_s + 0.9*so*th
                nah = work.tile([B, HS], f32, tag=f"{pfx}nah")
                nc.vector.scalar_tensor_tensor(nah[:], so, 0.9, th[:],
                                               OP.mult, OP.mult)
                nc.vector.scalar_tensor_tensor(h_s[:], h_s[:], 0.1, nah[:],
                                               OP.mult, OP.add)

            def transpose_to(dst_col, src_ap, rows):
                """src_ap [r<=128, rows<=128] -> PE transpose -> dst_col [rows, r]."""
                ps = psmall.tile([128, B], f32, tag="sm")
                r = src_ap.shape[0]
                nc.tensor.transpose(ps[:rows, :r], src_ap, ident[:r, :r])
                nc.scalar.copy(dst_col, ps[:rows, :r])

            for t in range(t_steps):
                # ---------------- attention-LSTM
                x_t = work.tile([128, KX, B], f32, tag="x_t")
                nc.sync.dma_start(x_t[:], din["pren"][t])
                ga = pgates.tile([B, GS], f32, tag="g")
                nc.tensor.matmul(ga[:], cw["ones_row"][:], cw["bias_tile"][:, 0, :],
                                 start=True, stop=False)
                for k in range(KX):
                    nc.tensor.matmul(ga[:], x_t[:, k, :], cw["wa_x"][:, k, :],
                                     start=False, stop=False)
                for k in range(KC):
                    nc.tensor.matmul(ga[:], ctxT[:, k, :], cw["wa_ctx"][:, k, :],
                                     start=False, stop=False)
                for k in range(KH):
                    nc.tensor.matmul(ga[:], ahT[:, k, :], cw["wa_h"][:, k, :],
                                     start=False, stop=(k == KH - 1))
                lstm_cell(ga, ac, ah_s, "a")

                # transpose my slice, bounce, allgather
                ahT_m = work.tile([128, B], f32, tag="ahT_m")
                transpose_to(ahT_m[:, 0:B], ah_s[:], 128)
                agA_i = dram.tile([128, B], f32, tag="agA_i")
                agA_o = dram.tile([NCORES, 128, B], f32, tag="agA_o")
                nc.sync.dma_start(agA_i[:], ahT_m[:])
                nc.gpsimd.collective_compute(
                    "AllGather", OP.bypass, replica_groups=rg,
                    ins=[agA_i[:]], outs=[agA_o[:]])
                for s in range(NCORES):
                    nc.sync.dma_start(ahT[:, s, :], agA_o[s])

                # ---------------- attention (batch-sharded)
                # location features from previous step's aw/w_cum
                im2 = attw.tile([2 * KS, BS, T_IN], f32, tag="im2")
                for c2 in range(2):
                    src = AP(tensor=conv_d, offset=c2 * BS * (T_IN + KS - 1),
                             ap=[[1, KS], [T_IN + KS - 1, BS], [1, T_IN]])
                    nc.sync.dma_start(im2[c2 * KS:(c2 + 1) * KS, :, :], src)
                pa = pbig.tile([ADIM, BS * T_IN], f32, tag="big")
                im2f = im2.rearrange("p b t -> p (b t)")
                nc.tensor.matmul(pa[:, 0:512], cw["wf_T"][:], im2f[:, 0:512],
                                 start=True, stop=True)
                nc.tensor.matmul(pa[:, 512:BS * T_IN], cw["wf_T"][:],
                                 im2f[:, 512:BS * T_IN], start=True, stop=True)
                papm = attw.tile([ADIM, BS, T_IN], f32, tag="papm")
                nc.vector.tensor_add(
                    papm[:], pa[:].rearrange("p (b t) -> p b t", b=BS),
                    cw["pm"][:])

                # pq batch-major for all batches: [B, ADIM] = sum_k ahT_k^T wq_k^T
                pq_ps = psmall.tile([B, ADIM], f32, tag="sm")
                for k in range(KH):
                    nc.tensor.matmul(pq_ps[:], ahT[:, k, :], cw["wq_T"][:, k, :],
                                     start=(k == 0), stop=(k == KH - 1))
                pq_bT = work.tile([B, ADIM], f32, tag="pq_bT")
                nc.scalar.copy(pq_bT[:], pq_ps[:])
                # select own 4 batches AND transpose: [ADIM, BS] = pq_bT^T @ sel
                pq_ps2 = psmall.tile([ADIM, BS], f32, tag="sm")
                nc.tensor.matmul(pq_ps2[:], pq_bT[:], sel_sb[:],
                                 start=True, stop=True)
                pq = work.tile([ADIM, BS], f32, tag="pq")
                nc.scalar.copy(pq[:], pq_ps2[:])

                tanh_o = attw.tile([ADIM, BS, T_IN], f32, tag="tanh_o")
                for b in range(BS):
                    nc.scalar.activation(tanh_o[:, b, :], papm[:, b, :],
                                         AF.Tanh, bias=pq[:, b:b + 1])
                en_ps = psmall.tile([BS, T_IN], f32, tag="sm")
                for b in range(BS):
                    nc.tensor.matmul(en_ps[:], cw["v_diag"][:, b, :],
                                     tanh_o[:, b, :],
                                     start=(b == 0), stop=(b == BS - 1))

                # forward attention in probability domain:
                # n = (p + shift(p)) * exp(energy); aw = n / sum(n)
                p_st = aw_buf[:, 1:T_IN + 1]
                p_sh = aw_buf[:, 0:T_IN]
                e_exp = attw.tile([BS, T_IN], f32, tag="e_exp")
                nc.scalar.activation(e_exp[:], en_ps[:], AF.Exp)
                q = attw.tile([BS, T_IN], f32, tag="q")
                nc.vector.tensor_add(q[:], p_st, p_sh)
                n_t = attw.tile([BS, T_IN], f32, tag="n_t")
                zsum = attw.tile([BS, 1], f32, tag="zsum")
                nc.vector.scalar_tensor_tensor(n_t[:], q[:], 1.0, e_exp[:],
                                               OP.mult, OP.mult,
                                               accum_out=zsum[:])
                rr = attw.tile([BS, 1], f32, tag="rr")
                nc.vector.reciprocal(rr[:], zsum[:])
                aw = attw.tile([BS, T_IN], f32, tag="aw")
                nc.vector.tensor_scalar_mul(aw[:], n_t[:], rr[:])
                nc.vector.tensor_copy(p_st, aw[:])

                nc.vector.tensor_add(w_cum[:], w_cum[:], aw[:])
                nc.sync.dma_start(aligns_o[t], aw[:])
                # write conv inputs for next step
                nc.sync.dma_start(
                    conv_d[0][:, KS // 2:KS // 2 + T_IN], aw[:])
                nc.sync.dma_start(
                    conv_d[1][:, KS // 2:KS // 2 + T_IN], w_cum[:])

                # aw^T into block-diagonal columns of awT_bd
                def diag_ap(kt, rows):
                    return AP(tensor=awT_bd.tensor,
                              offset=awT_bd.offset + BS * BS * kt,
                              ap=[[2 * BS * BS, rows], [BS + 1, BS]])
                transpose_to(diag_ap(0, 128), aw[:, 0:128], 128)
                transpose_to(diag_ap(1, T_IN - 128), aw[:, 128:T_IN], T_IN - 128)

                ctx_ps = pbig.tile([BS, E], f32, tag="big")
                for lo, hi in ((0, 512), (512, E)):
                    n_k = 0
                    for b in range(BS):
                        for kt in range(2):
                            nc.tensor.matmul(
                                ctx_ps[:, lo:hi],
                                awT_bd[:, kt, b, :],
                                mem_sb[:, b, kt, lo:hi],
                                start=(n_k == 0), stop=(n_k == 2 * BS - 1))
                            n_k += 1
                agB_i = dram.tile([BS, E], f32, tag="agB_i")
                agB_o = dram.tile([B, E], f32, tag="agB_o")
                ctx_sb = work.tile([BS, E], f32, tag="ctx_sb")
                nc.scalar.copy(ctx_sb[:], ctx_ps[:])
                nc.sync.dma_start(agB_i[:], ctx_sb[:])
                nc.gpsimd.collective_compute(
                    "AllGather", OP.bypass, replica_groups=rg,
                    ins=[agB_i[:]], outs=[agB_o[:]])
                ctxf = work.tile([B, E], f32, tag="ctxf")
                nc.sync.dma_start(ctxf[:], agB_o[:])
                for k in range(KC):
                    r = min(128, E - 128 * k)
                    transpose_to(ctxT[0:r, k, :], ctxf[:, 128 * k:128 * k + r], r)

                # ---------------- decoder LSTM
                gd = pgates.tile([B, GS], f32, tag="g")
                nc.tensor.matmul(gd[:], cw["ones_row"][:], cw["bias_tile"][:, 1, :],
                                 start=True, stop=False)
                for k in range(KH):
                    nc.tensor.matmul(gd[:], ahT[:, k, :], cw["wd_ah"][:, k, :],
                                     start=False, stop=False)
                for k in range(KC):
                    nc.tensor.matmul(gd[:], ctxT[:, k, :], cw["wd_ctx"][:, k, :],
                                     start=False, stop=False)
                for k in range(KH):
                    nc.tensor.matmul(gd[:], dhT[:, k, :], cw["wd_h"][:, k, :],
                                     start=False, stop=(k == KH - 1))
                lstm_cell(gd, dc, dh_s, "d")

                dhT_m = work.tile([128, B], f32, tag="dhT_m")
                transpose_to(dhT_m[:, 0:B], dh_s[:], 128)
                agC_i = dram.tile([128, B], f32, tag="agC_i")
                agC_o = dram.tile([NCORES, 128, B], f32, tag="agC_o")
                nc.sync.dma_start(agC_i[:], dhT_m[:])
                nc.gpsimd.collective_compute(
                    "AllGather", OP.bypass, replica_groups=rg,
                    ins=[agC_i[:]], outs=[agC_o[:]])
                for s in range(NCORES):
                    nc.sync.dma_start(dhT[:, s, :], agC_o[s])

                # ---------------- projection (own batches)
                # select own batch columns of dhT/ctxT via sel matmul is
                # expensive; instead project ALL batches (N=32 columns ->
                # out [?]) -- no: out partitions = M. We project with
                # stationary = activations for all B, then DMA only own cols?
                # Projection: out [NO? no -- we keep out [B?]]:
                # lhsT = dh/ctx tiles [128, B] full batch, rhs = wp [128, NO]
                # -> out [B, NO]; each core computes ALL batches (8x waste,
                # ~1.2us) then stores only its own 4 rows via sel DMA.
                pr = psmall.tile([B, NO], f32, tag="sm")
                nc.tensor.matmul(pr[:], cw["ones_row"][:], cw["wp_b"][:],
                                 start=True, stop=False)
                for k in range(KH):
                    nc.tensor.matmul(pr[:], dhT[:, k, :], cw["wp"][:, k, :],
                                     start=False, stop=False)
                for k in range(KC):
                    nc.tensor.matmul(pr[:], ctxT[:, k, :], cw["wp"][:, KH + k, :],
                                     start=False, stop=(k == KC - 1))
                # own rows: batch slice position differs per core; selected
                # via one extra matmul: own [BS, NO] = sel_sb^T... sel_sb is
                # [B, BS] (K=B? partitions=B=32) -> lhsT=sel_sb [32, 4],
                # rhs=pr [32, NO] -- rhs must be SBUF; pr is PSUM. Copy first.
                pr_sb = work.tile([B, NO], f32, tag="pr_sb")
                nc.scalar.copy(pr_sb[:], pr[:])
                own_ps = psmall.tile([BS, NO], f32, tag="sm")
                nc.tensor.matmul(own_ps[:], sel_sb[:], pr_sb[:],
                                 start=True, stop=True)
                own_sb = work.tile([BS, NO], f32, tag="own_sb")
                nc.scalar.copy(own_sb[:], own_ps[:])
                nc.sync.dma_start(mels_o[t], own_sb[:, 0:NMEL])
                nc.sync.dma_start(gates_o[t], own_sb[:, NMEL:NO])

    nc.compile()
    return nc, din


# ------------------------------------------------------------------- driver

def _assemble(results):
    mel = np.zeros((B, NMEL, T_OUT), np.float32)
    gate = np.zeros((B, T_OUT), np.float32)
    align = np.zeros((B, T_OUT, T_IN), np.float32)
    for c, r in enumerate(results):
        bsl = slice(BS * c, BS * (c + 1))
        mel[bsl] = r["mels_o"].transpose(1, 2, 0)
        gate[bsl] = r["gates_o"].T
        align[bsl] = r["aligns_o"].transpose(1, 0, 2)
    return mel, gate, align


def kernel(memory, decoder_inputs, params, _t_steps=T_OUT):
    global LAST_RESULT
    from concourse.bass_utils import run_bass_kernel_spmd

    in_maps = _prepare_in_maps(memory, decoder_inputs, params)
    sel = np.zeros((NCORES, B, BS), np.float32)
    for c in range(NCORES):
        for j in range(BS):
            sel[c, BS * c + j, j] = 1.0
    for c in range(NCORES):
        in_maps[c]["sel_b"] = sel[c]

    if _t_steps not in _CACHE:
        _CACHE[_t_steps] = _build(_t_steps)
    nc, _ = _CACHE[_t_steps]

    res = run_bass_kernel_spmd(nc, in_maps, core_ids=list(range(NCORES)),
                               trace=False)
    LAST_RESULT = res
    return _assemble(res.results)
